# revision 1
# baseline (speedup 1.0000x reference)
"""Binarized 3-layer MLP on 8 TRN2 NeuronCores (data-parallel over batch).

Computation (matching the reference):
    h1  = x @ sign(W1).T          x: [65536, 784] fp32, W1: [400, 784]
    h2  = sign(h1) @ sign(W2).T   W2: [200, 400]
    out = sign(h2) @ sign(W3).T   W3: [10, 200]

Strategy:
  - Batch sharded 8192 rows/core; weights replicated.
  - All activations kept feature-major (features on SBUF partitions), so the
    contraction dim of every layer is already on partitions: no transposes.
  - Layer 1 precision: x is split into two fp16 components (hi = fp16(x),
    lo = fp16(x - hi)) whose sum reproduces x to ~2^-23 relative (the PE
    handles fp16 subnormals exactly; HW-measured max err 4e-6 vs fp64 at
    K=128, same as fp32 matmul). Since sign(W1) is exactly +-1 in fp16, the
    two accumulated fp16 matmuls give fp32-quality h1 at full PE speed. The
    two components are concatenated along K (784*2 -> padded 1664 = 13
    k-tiles).
  - Layers 2/3: sign() outputs are exactly representable in bf16 and PSUM
    accumulates in fp32, so plain bf16 matmuls are exact.
  - The 400-row layer-1 output tiles as 128+128+128+16. The 16-row remainder
    ("m4") would waste a full-width matmul per k-tile, so chunks are processed
    in groups of 4 and the four 16-row matmuls are packed into one PSUM bank
    at partition strips 0/32/64/96 via tile_position col-tiling; the hardware
    runs matmuls in distinct 32-column groups concurrently. Accumulation uses
    memset-to-zero + start=False (accumulate-onto-zero == overwrite for any
    stale has_written state), which keeps interleaved strip accumulation
    correct. Layer 3 (M=10) outputs are packed the same way.
"""

import contextlib
import ctypes
import os
import sys
import types

import numpy as np
import ml_dtypes

import concourse.bacc as bacc
import concourse.mybir as mybir
import concourse.tile as tile
from concourse.bass_utils import run_bass_kernel_spmd


def _ensure_axon_hooks():
    """concourse's trace path imports antenv.axon_hooks, which this image
    lacks; register a ctypes-backed stand-in so trace=True (or a stray
    BASS_TRACE=1 in the environment) cannot crash the run."""
    try:
        import antenv.axon_hooks  # noqa: F401
        return
    except ImportError:
        pass

    so_path = "/opt/axon/libaxon_pjrt.so"
    hook = None
    if os.path.exists(so_path):
        try:
            lib = ctypes.CDLL(so_path)
            if hasattr(lib, "axon_start_nrt_profile"):
                lib.axon_start_nrt_profile.argtypes = [
                    ctypes.POINTER(ctypes.c_int64),
                    ctypes.c_size_t,
                ]
                lib.axon_start_nrt_profile.restype = ctypes.c_int64
                lib.axon_stop_nrt_profile.argtypes = [ctypes.c_char_p]
                lib.axon_stop_nrt_profile.restype = ctypes.c_int64

                @contextlib.contextmanager
                def _hook(output_dir, device_ids):
                    import jax

                    jax.devices()
                    if device_ids:
                        ids = (ctypes.c_int64 * len(device_ids))(*device_ids)
                        rc = lib.axon_start_nrt_profile(ids, len(device_ids))
                    else:
                        rc = lib.axon_start_nrt_profile(None, 0)
                    if rc != 0:
                        raise RuntimeError(f"axon_start_nrt_profile rc={rc}")
                    try:
                        yield
                    finally:
                        lib.axon_stop_nrt_profile(str(output_dir).encode())

                hook = _hook
        except OSError:
            pass

    mod = types.ModuleType("antenv.axon_hooks")
    mod.get_axon_ntff_profile_hook = lambda: hook
    mod.set_axon_ntff_profile_hook = lambda h: None
    sys.modules["antenv.axon_hooks"] = mod

    import concourse.bass_utils as _bu

    _bu.upload_artifacts = lambda tmpdir: tmpdir

BF16 = np.dtype(ml_dtypes.bfloat16)

NCORES = 8
B = 65536
BL = B // NCORES          # 8192 rows per core
D0, H1, H2, DO = 784, 400, 200, 10
CH = 512                  # batch columns per chunk (PSUM bank = 512 fp32)
NCH = BL // CH            # 16 chunks per core
GRP = 4                   # chunks per packing group
KT1 = 13                  # ceil(784*2 / 128) k-tiles for layer 1
K1P = KT1 * 128           # 1664 padded K for layer 1

_cache = {}


def _build():
    if "nc" in _cache:
        return _cache["nc"]

    f32 = mybir.dt.float32
    bf16 = mybir.dt.bfloat16
    f16 = mybir.dt.float16
    Sign = mybir.ActivationFunctionType.Sign

    nc = bacc.Bacc("TRN2", debug=False, num_devices=NCORES)

    d_x = nc.dram_tensor("xh", [NCH, 128, KT1, CH], f16, kind="ExternalInput").ap()
    # w1 split so the m1 slab (first matmuls) lands before the rest
    d_w1a = nc.dram_tensor("w1a", [128, KT1, 128], f16, kind="ExternalInput").ap()
    d_w1b = nc.dram_tensor("w1b", [128, KT1, H1 - 128], f16, kind="ExternalInput").ap()
    # w2 k-blocks 0..2 are features 0:384; block 3 holds features 384:400
    # replicated at partition strips 0/32/64/96 (matches packed a1_3 layout).
    d_w2 = nc.dram_tensor("w2", [128, 4, H2], bf16, kind="ExternalInput").ap()
    d_w3 = nc.dram_tensor("w3", [128, 2, DO], bf16, kind="ExternalInput").ap()
    d_out = nc.dram_tensor("out", [NCH, DO, CH], f32, kind="ExternalOutput").ap()

    m1sz = [128, 128, 128]     # full-width layer-1 m-tiles (m4 packed separately)
    k2sz = [128, 128, 128]     # layer-2 full k-tiles (k4=16 handled via strips)
    m2sz = [128, 72]
    k3sz = [128, 72]

    with tile.TileContext(nc) as tc:
        with (
            tc.tile_pool(name="wp", bufs=1) as wp,
            tc.tile_pool(name="xp", bufs=6) as xp,
            tc.tile_pool(name="ap_", bufs=2) as apool,
            tc.tile_pool(name="a2p", bufs=2) as a2pool,
            tc.tile_pool(name="op", bufs=2) as op,
            tc.tile_pool(name="ps1p", bufs=1, space="PSUM") as ps1p,
            tc.tile_pool(name="ps2p", bufs=1, space="PSUM") as ps2p,
            tc.tile_pool(name="pspk", bufs=2, space="PSUM") as pspk,
        ):
            w1a = wp.tile([128, KT1, 128], f16, name="w1a")
            w1b = wp.tile([128, KT1, H1 - 128], f16, name="w1b")
            w2sb = wp.tile([128, 4, H2], bf16, name="w2sb")
            w3sb = wp.tile([128, 2, DO], bf16, name="w3sb")
            nc.sync.dma_start(out=w1a[:], in_=d_w1a)

            def w1_slice(k, m_off, m_sz):
                if m_off == 0:
                    return w1a[:, k, 0:m_sz]
                return w1b[:, k, m_off - 128 : m_off - 128 + m_sz]

            def layer1_m123(xch):
                """Full-width layer-1 m-tiles; returns [a1_m0, a1_m1, a1_m2].

                The last k-tile holds only 32 real K-rows, replicated host-side
                at partition strips 0/32/64 so the three m-tiles' tail matmuls
                run concurrently in distinct PE row-groups."""
                a1 = []
                pss = []
                for m in range(3):
                    ps = ps1p.tile(
                        [128, CH], f32, name=f"ps1_{m}", bufs=(2 if m == 0 else 1)
                    )
                    for k in range(KT1 - 1):
                        nc.tensor.matmul(
                            ps[:],
                            w1_slice(k, m * 128, 128),
                            xch[:, k, :],
                            start=(k == 0),
                            stop=False,
                        )
                    pss.append(ps)
                kl = KT1 - 1
                for m in range(3):
                    s = 32 * m
                    lhsT = (
                        w1a[s : s + 32, kl, 0:128]
                        if m == 0
                        else w1b[s : s + 32, kl, (m - 1) * 128 : m * 128]
                    )
                    nc.tensor.matmul(
                        pss[m][:],
                        lhsT,
                        xch[s : s + 32, kl, :],
                        start=False,
                        stop=True,
                        tile_position=(s, 0),
                    )
                for m in range(3):
                    at = apool.tile([128, CH], bf16, name=f"a1_{m}")
                    nc.scalar.activation(at[:], pss[m][:], Sign)
                    a1.append(at)
                return a1

            def layer2(jj, a1m, a13p):
                """Layer 2 for chunk jj of the group; a13p is the packed
                (4-strip) a1 remainder tile. Returns [a2_m0, a2_m1]."""
                a2 = [None, None]
                # alternate m order per chunk so consecutive chunks' same-m
                # groups are further apart (ps2 banks are single-buffered)
                for m in ((0, 1) if jj % 2 == 0 else (1, 0)):
                    sz = m2sz[m]
                    ps = ps2p.tile([sz, CH], f32, name=f"ps2_{m}")
                    for k in range(3):
                        nc.tensor.matmul(
                            ps[:],
                            w2sb[:, k, m * 128 : m * 128 + sz],
                            a1m[k][:],
                            start=(k == 0),
                            stop=False,
                        )
                    s = 32 * jj
                    nc.tensor.matmul(
                        ps[:],
                        w2sb[s : s + 16, 3, m * 128 : m * 128 + sz],
                        a13p[s : s + 16, :],
                        start=False,
                        stop=True,
                        tile_position=(s, 0),
                    )
                    at = a2pool.tile([sz, CH], bf16, name=f"a2_{jj}_{m}")
                    nc.scalar.activation(at[:], ps[:], Sign)
                    a2[m] = at
                return a2

            # HAM/P-state pre-warm: dummy matmuls on a scratch tile keep the
            # PE busy during the initial weight/x DMA wait so the first real
            # matmuls run at full clock (the activity window is ~3.4us).
            warm = wp.tile([128, 64], f16, name="warm")
            nc.vector.memset(warm[:], 1.0)
            wps = pspk.tile([64, 64], f32, name="wps", tag="pack")
            for _ in range(48):
                nc.tensor.matmul(wps[:], warm[:, 0:64], warm[:], start=True, stop=True)

            for g in range(NCH // GRP):
                xchs = []
                for jj in range(GRP):
                    xch = xp.tile([128, KT1, CH], f16, name="xch")
                    nc.sync.dma_start(out=xch[:], in_=d_x[g * GRP + jj])
                    xchs.append(xch)
                    if g == 0 and jj == 0:
                        nc.sync.dma_start(out=w1b[:], in_=d_w1b)
                    if g == 0 and jj == 1:
                        nc.sync.dma_start(out=w2sb[:], in_=d_w2)
                        nc.sync.dma_start(out=w3sb[:], in_=d_w3)

                # packed m4 PSUM bank: strips [32jj : 32jj+16] per chunk
                ps4 = pspk.tile([128, CH], f32, name="ps4", tag="pack")
                nc.vector.memset(ps4[:], 0.0)

                a1s = [None] * GRP
                a1s[0] = layer1_m123(xchs[0])
                a1s[1] = layer1_m123(xchs[1])

                # m4 packed: 4 col-tiled strips, interleaved for concurrency
                for k in range(KT1):
                    kr = 32 if k == KT1 - 1 else 128  # real rows in tail tile
                    for jj in range(GRP):
                        s = 32 * jj
                        nc.tensor.matmul(
                            ps4[s : s + 16, :],
                            w1_slice(k, 384, 16)[0:kr],
                            xchs[jj][0:kr, k, :],
                            start=False,
                            stop=(k == KT1 - 1),
                            tile_position=(0, s),
                        )
                a13p = apool.tile([128, CH], bf16, name="a13p")
                nc.scalar.activation(a13p[:], ps4[:], Sign)

                a2s = [None] * GRP
                a2s[0] = layer2(0, a1s[0], a13p)
                a2s[1] = layer2(1, a1s[1], a13p)
                a1s[2] = layer1_m123(xchs[2])
                a2s[2] = layer2(2, a1s[2], a13p)
                a1s[3] = layer1_m123(xchs[3])
                a2s[3] = layer2(3, a1s[3], a13p)

                # layer 3, packed into one PSUM bank at strips [32jj:32jj+10]
                ps3 = pspk.tile([128, CH], f32, name="ps3", tag="pack")
                nc.vector.memset(ps3[:], 0.0)
                for k in range(2):
                    ks = k3sz[k]
                    for jj in range(GRP):
                        s = 32 * jj
                        nc.tensor.matmul(
                            ps3[s : s + DO, :],
                            w3sb[0:ks, k, :],
                            a2s[jj][k][0:ks, :],
                            start=False,
                            stop=(k == 1),
                            tile_position=(0, s),
                        )
                osb = op.tile([128, CH], f32, name="osb")
                nc.vector.tensor_copy(osb[:], ps3[:])
                for jj in range(GRP):
                    s = 32 * jj
                    nc.sync.dma_start(
                        out=d_out[g * GRP + jj], in_=osb[s : s + DO, :]
                    )

    nc.compile()
    _cache["nc"] = nc
    return nc


def _prep_weights(W1, W2, W3):
    # [K, M] layouts, K on partitions, padded so K-tiles are uniform 128.
    w1T = np.sign(W1).T.astype(np.float32)  # [784, 400]
    w1cat = np.concatenate(
        [w1T, w1T, np.zeros((K1P - 2 * D0, H1), np.float32)], axis=0
    )  # [1664, 400]
    # replicate the 32-row K-tail at partition strips 32/64 of the last
    # k-tile (for row-packed concurrent tail matmuls)
    w1cat[1568:1600] = w1cat[1536:1568]
    w1cat[1600:1632] = w1cat[1536:1568]
    w1h = np.ascontiguousarray(
        w1cat.reshape(KT1, 128, H1).transpose(1, 0, 2)
    ).astype(np.float16)  # [128, 13, 400]
    w1ha = np.ascontiguousarray(w1h[:, :, 0:128])
    w1hb = np.ascontiguousarray(w1h[:, :, 128:H1])

    w2T = np.sign(W2).T.astype(np.float32)  # [400, 200]
    w2h = np.zeros((128, 4, H2), np.float32)
    for k in range(3):
        w2h[:, k, :] = w2T[k * 128 : (k + 1) * 128]
    for jj in range(GRP):
        w2h[32 * jj : 32 * jj + 16, 3, :] = w2T[384:400]
    w2h = w2h.astype(BF16)

    w3T = np.sign(W3).T.astype(np.float32)  # [200, 10]
    w3h = np.zeros((128, 2, DO), np.float32)
    w3h[:, 0, :] = w3T[0:128]
    w3h[0:72, 1, :] = w3T[128:200]
    w3h = w3h.astype(BF16)
    return w1ha, w1hb, w2h, w3h


def _prep_x_core(xc):
    # xc: [8192, 784] fp32 -> [16, 128, 13, 512] fp16 (hi/lo along K)
    xt = np.ascontiguousarray(xc.T.astype(np.float32))  # [784, 8192]
    hi = xt.astype(np.float16)
    lo = (xt - hi.astype(np.float32)).astype(np.float16)
    x2 = np.concatenate(
        [hi, lo, np.zeros((K1P - 2 * D0, BL), np.float16)], axis=0
    )  # [1664, 8192]
    x2[1568:1600] = x2[1536:1568]
    x2[1600:1632] = x2[1536:1568]
    return np.ascontiguousarray(
        x2.reshape(KT1, 128, NCH, CH).transpose(2, 1, 0, 3)
    )  # [16, 128, 13, 512]


def kernel(x, W1, W2, W3, _trace=False, **_kw):
    nc = _build()
    w1ha, w1hb, w2h, w3h = _prep_weights(
        np.asarray(W1, np.float32), np.asarray(W2, np.float32), np.asarray(W3, np.float32)
    )
    x = np.asarray(x, np.float32).reshape(B, D0)

    in_maps = []
    for c in range(NCORES):
        in_maps.append(
            {
                "xh": _prep_x_core(x[c * BL : (c + 1) * BL]),
                "w1a": w1ha,
                "w1b": w1hb,
                "w2": w2h,
                "w3": w3h,
            }
        )

    _ensure_axon_hooks()
    res = run_bass_kernel_spmd(nc, in_maps, core_ids=list(range(NCORES)), trace=_trace)

    out = np.empty((B, DO), np.float32)
    for c in range(NCORES):
        oc = res.results[c]["out"]  # [16, 10, 512]
        out[c * BL : (c + 1) * BL] = oc.transpose(0, 2, 1).reshape(BL, DO)
    if _trace:
        _cache["last_results"] = res
    return out



# revision 4
# speedup vs baseline: 1.2606x; 1.2606x over previous
"""Binarized 3-layer MLP on 8 TRN2 NeuronCores (data-parallel over batch).

Computation (matching the reference):
    h1  = x @ sign(W1).T          x: [65536, 784] fp32, W1: [400, 784]
    h2  = sign(h1) @ sign(W2).T   W2: [200, 400]
    out = sign(h2) @ sign(W3).T   W3: [10, 200]

Strategy (v2):
  - Batch sharded 8192 rows/core; weights replicated.
  - Layer 1 precision: x split as hi = e4m3(x) plus lo = fp16(x - hi).
    The hi pass runs as fp8 DoubleRow matmuls (2 K-rows per PE cell,
    K=256 per matmul, ~1.5-2x bf16 rate); the lo pass runs as fp16
    matmuls. Combined representation error ~2^-15 relative, giving
    rel err ~8e-3 on the fixed-seed inputs (sign-flip propagation
    through the two binarized layers) vs the 2e-2 gate.
  - K tail (rows 768:784): hi(e4m3->fp16 exact) and lo stacked as one
    32-row fp16 tile, replicated at partition strips 0/32/64 so the
    three m-tiles' tail matmuls run concurrently in one window.
  - The 400-row layer-1 output tiles as 128+128+128+16. The 16-row
    remainder (m4) is col-packed: groups of 4 chunks run their m4
    matmuls in distinct 32-column PE groups concurrently.
  - Layer 2 (K=400): fp8 DoubleRow K=256 + normal fp8 K=128 + per-chunk
    16-row strip tail from the packed m4 signs.
  - Layer 3 (K=200): one fp8 DoubleRow matmul per chunk; the a2 pair
    tile holds features 0:128 at pair 0 and 128:200 at pair 1 with
    zeroed weights over the 72:128 garbage partitions.
  - Sign() outputs are exactly representable in e4m3, and PSUM
    accumulates fp32, so layers 2/3 are exact.
"""

import contextlib
import ctypes
import os
import sys
import types

import numpy as np
import ml_dtypes

import concourse.bacc as bacc
import concourse.mybir as mybir
import concourse.tile as tile
from concourse.bass_utils import run_bass_kernel_spmd


def _ensure_axon_hooks():
    """concourse's trace path imports antenv.axon_hooks, which this image
    lacks; register a ctypes-backed stand-in so trace=True (or a stray
    BASS_TRACE=1 in the environment) cannot crash the run."""
    try:
        import antenv.axon_hooks  # noqa: F401
        return
    except ImportError:
        pass

    so_path = "/opt/axon/libaxon_pjrt.so"
    hook = None
    if os.path.exists(so_path):
        try:
            lib = ctypes.CDLL(so_path)
            if hasattr(lib, "axon_start_nrt_profile"):
                lib.axon_start_nrt_profile.argtypes = [
                    ctypes.POINTER(ctypes.c_int64),
                    ctypes.c_size_t,
                ]
                lib.axon_start_nrt_profile.restype = ctypes.c_int64
                lib.axon_stop_nrt_profile.argtypes = [ctypes.c_char_p]
                lib.axon_stop_nrt_profile.restype = ctypes.c_int64

                @contextlib.contextmanager
                def _hook(output_dir, device_ids):
                    import jax

                    jax.devices()
                    if device_ids:
                        ids = (ctypes.c_int64 * len(device_ids))(*device_ids)
                        rc = lib.axon_start_nrt_profile(ids, len(device_ids))
                    else:
                        rc = lib.axon_start_nrt_profile(None, 0)
                    if rc != 0:
                        raise RuntimeError(f"axon_start_nrt_profile rc={rc}")
                    try:
                        yield
                    finally:
                        lib.axon_stop_nrt_profile(str(output_dir).encode())

                hook = _hook
        except OSError:
            pass

    mod = types.ModuleType("antenv.axon_hooks")
    mod.get_axon_ntff_profile_hook = lambda: hook
    mod.set_axon_ntff_profile_hook = lambda h: None
    sys.modules["antenv.axon_hooks"] = mod

    import concourse.bass_utils as _bu

    _bu.upload_artifacts = lambda tmpdir: tmpdir


E4M3 = np.dtype(ml_dtypes.float8_e4m3)

NCORES = 8
B = 65536
BL = B // NCORES          # 8192 rows per core
D0, H1, H2, DO = 784, 400, 200, 10
CH = 512                  # batch columns per chunk (PSUM bank = 512 fp32)
NCH = BL // CH            # 16 chunks per core
GRP = 4                   # chunks per m4 packing group
KH = 3                    # hi-pass DoubleRow k-tiles (K=256 each, 768 rows)
KL = 6                    # lo-pass fp16 k-tiles (K=128 each, 768 rows)
NWARM = 90                # PE warmup matmuls (HAM ramp + DMA-wait cover)

_cache = {}


def _build():
    if "nc" in _cache:
        return _cache["nc"]

    f32 = mybir.dt.float32
    f16 = mybir.dt.float16
    f8 = mybir.dt.float8e4
    DR = mybir.MatmulPerfMode.DoubleRow
    Sign = mybir.ActivationFunctionType.Sign

    nc = bacc.Bacc("TRN2", debug=False, num_devices=NCORES)

    # x hi plane: [chunk][p, k, pair, col], K-row r = 256k + 128i + p
    d_xhi = nc.dram_tensor("xhi", [NCH, 128, KH, 2, CH], f8, kind="ExternalInput").ap()
    # x lo plane + tail: k 0:6 lo (r = 128k + p); k=6 = 32-row fp16 tail
    # (hi[768:784] ++ lo[768:784]) replicated at partition strips 0/32/64
    d_xlt = nc.dram_tensor("xlt", [NCH, 128, KL + 1, CH], f16, kind="ExternalInput").ap()
    d_w1h = nc.dram_tensor("w1h", [128, KH, 2, H1], f8, kind="ExternalInput").ap()
    d_w1l = nc.dram_tensor("w1l", [128, KL + 1, H1], f16, kind="ExternalInput").ap()
    d_w2a = nc.dram_tensor("w2a", [128, 2, 256], f8, kind="ExternalInput").ap()
    d_w2n = nc.dram_tensor("w2n", [128, H2], f8, kind="ExternalInput").ap()
    d_w2t = nc.dram_tensor("w2t", [128, H2], f8, kind="ExternalInput").ap()
    d_w3 = nc.dram_tensor("w3", [128, 2, 16], f8, kind="ExternalInput").ap()
    d_out = nc.dram_tensor("out", [NCH, DO, CH], f32, kind="ExternalOutput").ap()

    with tile.TileContext(nc) as tc:
        with (
            tc.tile_pool(name="wp", bufs=1) as wp,
            tc.tile_pool(name="xp", bufs=8) as xp,
            tc.tile_pool(name="ap_", bufs=2) as apool,
            tc.tile_pool(name="a2p", bufs=2) as a2pool,
            tc.tile_pool(name="op", bufs=2) as op,
            tc.tile_pool(name="ps1p", bufs=1, space="PSUM") as ps1p,
            tc.tile_pool(name="ps2p", bufs=1, space="PSUM") as ps2p,
            tc.tile_pool(name="pspk", bufs=2, space="PSUM") as pspk,
        ):
            w1h = wp.tile([128, KH, 2, H1], f8, name="w1h")
            w1l = wp.tile([128, KL + 1, H1], f16, name="w1l")
            w2a = wp.tile([128, 2, 256], f8, name="w2a")
            w2n = wp.tile([128, H2], f8, name="w2n")
            w2t = wp.tile([128, H2], f8, name="w2t")
            w3 = wp.tile([128, 2, 16], f8, name="w3")
            nc.sync.dma_start(out=w1h[:], in_=d_w1h)

            # HAM/P-state pre-warm: dummy matmuls keep the PE busy during the
            # initial weight/x DMA wait so the first real matmuls run at full
            # clock (the activity window is ~3.4us).
            warm = wp.tile([128, 64], f16, name="warm")
            nc.vector.memset(warm[:], 1.0)
            wps = pspk.tile([64, 64], f32, name="wps", tag="pack")
            for _ in range(NWARM):
                nc.tensor.matmul(wps[:], warm[:, 0:64], warm[:], start=True, stop=True)

            def layer1(xhi, xlt, last=False):
                """Full-width layer-1 m-tiles; returns (a1p, a1m2).

                a1p[:, 0/1, :] = sign(h1) features 0:128 / 128:256 (e4m3);
                a1m2 = features 256:384."""
                pss = []
                for m in range(3):
                    ps = ps1p.tile(
                        [128, CH], f32, name=f"ps1_{m}", bufs=(2 if m == 0 else 1)
                    )
                    for k in range(KH):
                        nc.tensor.matmul(
                            ps[:],
                            w1h[:, k, :, m * 128 : (m + 1) * 128],
                            xhi[:, k, :, :],
                            start=(k == 0),
                            stop=False,
                            perf_mode=DR,
                        )
                    for k in range(KL):
                        nc.tensor.matmul(
                            ps[:],
                            w1l[:, k, m * 128 : (m + 1) * 128],
                            xlt[:, k, :],
                            start=False,
                            stop=False,
                        )
                    if last:
                        # tail immediately per m-tile so Sign fires early and
                        # the final L2/L3 chain shortens the kernel epilogue
                        s = 32 * m
                        nc.tensor.matmul(
                            ps[:],
                            w1l[s : s + 32, KL, m * 128 : (m + 1) * 128],
                            xlt[s : s + 32, KL, :],
                            start=False,
                            stop=True,
                        )
                    pss.append(ps)
                if not last:
                    # shared 32-row fp16 tail window: 3 m-tiles at partition
                    # strips 0/32/64 run concurrently in distinct row groups
                    for m in range(3):
                        s = 32 * m
                        nc.tensor.matmul(
                            pss[m][:],
                            w1l[s : s + 32, KL, m * 128 : (m + 1) * 128],
                            xlt[s : s + 32, KL, :],
                            start=False,
                            stop=True,
                        )
                a1p = apool.tile([128, 2, CH], f8, name="a1p")
                a1m2 = apool.tile([128, CH], f8, name="a1m2")
                nc.scalar.activation(a1p[:, 0, :], pss[0][:], Sign)
                nc.scalar.activation(a1p[:, 1, :], pss[1][:], Sign)
                nc.scalar.activation(a1m2[:], pss[2][:], Sign)
                return a1p, a1m2

            def m4_group(xhis, xlts):
                """Packed m4 (features 384:400) for 4 chunks: matmuls at col
                strips 0/32/64/96 run concurrently. Returns a13p (e4m3)."""
                ps4 = pspk.tile([128, CH], f32, name="ps4", tag="pack")
                nc.vector.memset(ps4[:], 0.0)
                for k in range(KH):
                    for i in range(2):
                        for jj in range(GRP):
                            s = 32 * jj
                            nc.tensor.matmul(
                                ps4[s : s + 16, :],
                                w1h[:, k, i, 384:400],
                                xhis[jj][:, k, i, :],
                                start=False,
                                stop=False,
                                tile_position=(0, s),
                            )
                for k in range(KL):
                    for jj in range(GRP):
                        s = 32 * jj
                        nc.tensor.matmul(
                            ps4[s : s + 16, :],
                            w1l[:, k, 384:400],
                            xlts[jj][:, k, :],
                            start=False,
                            stop=False,
                            tile_position=(0, s),
                        )
                for jj in range(GRP):
                    s = 32 * jj
                    nc.tensor.matmul(
                        ps4[s : s + 16, :],
                        w1l[0:32, KL, 384:400],
                        xlts[jj][0:32, KL, :],
                        start=False,
                        stop=(jj == GRP - 1),
                        tile_position=(0, s),
                    )
                a13p = apool.tile([128, CH], f8, name="a13p")
                nc.scalar.activation(a13p[:], ps4[:], Sign)
                return a13p

            def layer23(jj, a1p, a1m2, a13p, g):
                """Layer 2 (DR K=256 + normal K=128 + 16-row strip tail) and
                layer 3 (one DR matmul), then output copy + DMA."""
                s = 32 * jj
                a2 = a2pool.tile([128, 2, CH], f8, name="a2")
                pss2 = []
                for m, msz in ((0, 128), (1, 72)):
                    mo = m * 128
                    ps = ps2p.tile([msz, CH], f32, name=f"ps2_{m}")
                    nc.tensor.matmul(
                        ps[:],
                        w2a[:, :, mo : mo + msz],
                        a1p[:],
                        start=True,
                        stop=False,
                        perf_mode=DR,
                    )
                    nc.tensor.matmul(
                        ps[:],
                        w2n[:, mo : mo + msz],
                        a1m2[:],
                        start=False,
                        stop=False,
                    )
                    nc.tensor.matmul(
                        ps[:],
                        w2t[s : s + 16, mo : mo + msz],
                        a13p[s : s + 16, :],
                        start=False,
                        stop=True,
                        tile_position=(s, 0),
                    )
                    pss2.append(ps)
                nc.scalar.activation(a2[:, 0, :], pss2[0][:], Sign)
                nc.scalar.activation(a2[0:72, 1, :], pss2[1][:], Sign)

                ps3 = pspk.tile([128, CH], f32, name="ps3", tag="pack")
                nc.tensor.matmul(
                    ps3[0:DO, :],
                    w3[:, :, 0:DO],
                    a2[:],
                    start=True,
                    stop=True,
                    perf_mode=DR,
                )
                osb = op.tile([16, CH], f32, name="osb")
                nc.vector.tensor_copy(osb[0:DO, :], ps3[0:DO, :])
                nc.sync.dma_start(out=d_out[g * GRP + jj], in_=osb[0:DO, :])

            # zero the a2 pair-1 garbage partitions once per rotating buffer
            # (w3 is also zero there; this guards against NaN/Inf x 0)
            for _ in range(2):
                a2z = a2pool.tile([128, 2, CH], f8, name="a2")
                nc.vector.memset(a2z[64:128, 1, :], 0.0)

            def dma_group(g):
                xhis, xlts = [], []
                for jj in range(GRP):
                    xhi = xp.tile([128, KH, 2, CH], f8, name="xhi")
                    xlt = xp.tile([128, KL + 1, CH], f16, name="xlt")
                    nc.sync.dma_start(out=xhi[:], in_=d_xhi[g * GRP + jj])
                    nc.sync.dma_start(out=xlt[:], in_=d_xlt[g * GRP + jj])
                    xhis.append(xhi)
                    xlts.append(xlt)
                    if g == 0 and jj == 0:
                        nc.sync.dma_start(out=w1l[:], in_=d_w1l)
                    if g == 0 and jj == 1:
                        nc.sync.dma_start(out=w2a[:], in_=d_w2a)
                        nc.sync.dma_start(out=w2n[:], in_=d_w2n)
                        nc.sync.dma_start(out=w2t[:], in_=d_w2t)
                        nc.sync.dma_start(out=w3[:], in_=d_w3)
                return xhis, xlts

            nxt = dma_group(0)
            for g in range(NCH // GRP):
                xhis, xlts = nxt
                if g + 1 < NCH // GRP:
                    nxt = dma_group(g + 1)

                last = g == NCH // GRP - 1
                a1s = [None] * GRP
                a1s[0] = layer1(xhis[0], xlts[0])
                a1s[1] = layer1(xhis[1], xlts[1])
                a13p = m4_group(xhis, xlts)
                layer23(0, *a1s[0], a13p, g)
                layer23(1, *a1s[1], a13p, g)
                a1s[2] = layer1(xhis[2], xlts[2])
                layer23(2, *a1s[2], a13p, g)
                a1s[3] = layer1(xhis[3], xlts[3], last=last)
                layer23(3, *a1s[3], a13p, g)

    nc.compile()
    _cache["nc"] = nc
    return nc


def _prep_weights(W1, W2, W3):
    s1 = np.sign(W1).T.astype(np.float32)  # [784, 400]
    w1h = np.ascontiguousarray(
        s1[:768].reshape(KH, 2, 128, H1).transpose(2, 0, 1, 3)
    ).astype(E4M3)  # [128, 3, 2, 400]
    w1l = np.zeros((128, KL + 1, H1), np.float16)
    w1l[:, 0:KL, :] = s1[:768].reshape(KL, 128, H1).transpose(1, 0, 2)
    trip = np.concatenate([s1[768:784], s1[768:784]], axis=0)  # [32, 400]
    for m in range(3):
        w1l[32 * m : 32 * m + 32, KL, :] = trip

    s2 = np.sign(W2).T.astype(np.float32)  # [400, 200]
    w2a = np.zeros((128, 2, 256), np.float32)
    w2a[:, 0, 0:H2] = s2[0:128]
    w2a[:, 1, 0:H2] = s2[128:256]
    w2n = s2[256:384]  # [128, 200]
    w2t = np.zeros((128, H2), np.float32)
    for jj in range(GRP):
        w2t[32 * jj : 32 * jj + 16] = s2[384:400]

    s3 = np.sign(W3).T.astype(np.float32)  # [200, 10]
    w3 = np.zeros((128, 2, 16), np.float32)
    w3[:, 0, 0:DO] = s3[0:128]
    w3[0:72, 1, 0:DO] = s3[128:200]

    return (
        w1h,
        w1l,
        w2a.astype(E4M3),
        np.ascontiguousarray(w2n).astype(E4M3),
        w2t.astype(E4M3),
        w3.astype(E4M3),
    )


def _prep_x_core(xc):
    # xc: [8192, 784] fp32 -> hi e4m3 [16, 128, 3, 2, 512], lo+tail fp16
    # [16, 128, 7, 512]
    xt = np.ascontiguousarray(xc.T.astype(np.float32))  # [784, 8192]
    hi8 = xt.astype(E4M3)
    lo = (xt - hi8.astype(np.float32)).astype(np.float16)  # [784, 8192]
    xhi = np.ascontiguousarray(
        hi8[:768].reshape(KH, 2, 128, NCH, CH).transpose(3, 2, 0, 1, 4)
    )  # [16, 128, 3, 2, 512]
    xlt = np.zeros((NCH, 128, KL + 1, CH), np.float16)
    xlt[:, :, 0:KL, :] = lo[:768].reshape(KL, 128, NCH, CH).transpose(2, 1, 0, 3)
    hi16 = hi8[768:784].astype(np.float16)  # exact
    tail = np.zeros((128, BL), np.float16)
    for m in range(3):
        tail[32 * m : 32 * m + 16] = hi16
        tail[32 * m + 16 : 32 * m + 32] = lo[768:784]
    xlt[:, :, KL, :] = tail.reshape(128, NCH, CH).transpose(1, 0, 2)
    return xhi, np.ascontiguousarray(xlt)


def kernel(x, W1, W2, W3, _trace=False, **_kw):
    nc = _build()
    w1h, w1l, w2a, w2n, w2t, w3 = _prep_weights(
        np.asarray(W1, np.float32), np.asarray(W2, np.float32), np.asarray(W3, np.float32)
    )
    x = np.asarray(x, np.float32).reshape(B, D0)

    in_maps = []
    for c in range(NCORES):
        xhi, xlt = _prep_x_core(x[c * BL : (c + 1) * BL])
        in_maps.append(
            {
                "xhi": xhi,
                "xlt": xlt,
                "w1h": w1h,
                "w1l": w1l,
                "w2a": w2a,
                "w2n": w2n,
                "w2t": w2t,
                "w3": w3,
            }
        )

    _ensure_axon_hooks()
    res = run_bass_kernel_spmd(nc, in_maps, core_ids=list(range(NCORES)), trace=_trace)

    out = np.empty((B, DO), np.float32)
    for c in range(NCORES):
        oc = res.results[c]["out"]  # [16, 10, 512]
        out[c * BL : (c + 1) * BL] = oc.transpose(0, 2, 1).reshape(BL, DO)
    if _trace:
        _cache["last_results"] = res
    return out


# revision 7
# speedup vs baseline: 1.3517x; 1.0723x over previous
"""Binarized 3-layer MLP on 8 TRN2 NeuronCores (data-parallel over batch).

Computation (matching the reference):
    h1  = x @ sign(W1).T          x: [65536, 784] fp32, W1: [400, 784]
    h2  = sign(h1) @ sign(W2).T   W2: [200, 400]
    out = sign(h2) @ sign(W3).T   W3: [10, 200]

Strategy (v3):
  - Batch sharded 8192 rows/core; weights replicated.
  - Layer 1 precision: x split as hi = e4m3(x) plus lo = fp16(x - hi).
    The hi pass runs as fp8 DoubleRow matmuls (2 K-rows per PE cell,
    K=256 per matmul); the lo pass runs as fp16 matmuls. Combined
    representation error ~2^-15 relative -> rel err ~8e-3 via sign-flip
    propagation, vs the 2e-2 gate.
  - K tail (rows 768:784): hi(e4m3->fp16 exact) and lo stacked as one
    32-row fp16 tile, replicated at partition strips 0/32/64 so the
    three m-tiles' tail matmuls run concurrently in one window.
  - m4 (h1 features 384:400): col-packed, groups of 4 chunks run their
    m4 matmuls in distinct 32-column PE groups concurrently.
  - Layer 2 (K=400): two fp8 DoubleRow matmuls per m-tile. The second
    pairs features 256:384 (a1x pair 0) with the packed m4 signs
    (a1x pair 1 = copy of a13p); per-chunk-position weight tiles w2b[jj]
    zero out all but that chunk's 16-row strip.
  - Layer 3 (K=200): one fp8 DoubleRow matmul per chunk; the a2 pair
    tile holds features 0:128 at pair 0 and 128:200 at pair 1 with
    zeroed weights over the 72:128 garbage partitions.
  - Sign() outputs are exactly representable in e4m3, and PSUM
    accumulates fp32, so layers 2/3 are exact.
  - Issue schedule is software-pipelined: L2 lags L1 by one chunk and
    L3 by two, so Sign (scalar engine) latency is hidden under the next
    chunk's matmul streams, across group boundaries.
"""

import contextlib
import ctypes
import os
import sys
import types

import numpy as np
import ml_dtypes

import concourse.bacc as bacc
import concourse.mybir as mybir
import concourse.tile as tile
from concourse.bass_utils import run_bass_kernel_spmd


def _ensure_axon_hooks():
    """concourse's trace path imports antenv.axon_hooks, which this image
    lacks; register a ctypes-backed stand-in so trace=True (or a stray
    BASS_TRACE=1 in the environment) cannot crash the run."""
    try:
        import antenv.axon_hooks  # noqa: F401
        return
    except ImportError:
        pass

    so_path = "/opt/axon/libaxon_pjrt.so"
    hook = None
    if os.path.exists(so_path):
        try:
            lib = ctypes.CDLL(so_path)
            if hasattr(lib, "axon_start_nrt_profile"):
                lib.axon_start_nrt_profile.argtypes = [
                    ctypes.POINTER(ctypes.c_int64),
                    ctypes.c_size_t,
                ]
                lib.axon_start_nrt_profile.restype = ctypes.c_int64
                lib.axon_stop_nrt_profile.argtypes = [ctypes.c_char_p]
                lib.axon_stop_nrt_profile.restype = ctypes.c_int64

                @contextlib.contextmanager
                def _hook(output_dir, device_ids):
                    import jax

                    jax.devices()
                    if device_ids:
                        ids = (ctypes.c_int64 * len(device_ids))(*device_ids)
                        rc = lib.axon_start_nrt_profile(ids, len(device_ids))
                    else:
                        rc = lib.axon_start_nrt_profile(None, 0)
                    if rc != 0:
                        raise RuntimeError(f"axon_start_nrt_profile rc={rc}")
                    try:
                        yield
                    finally:
                        lib.axon_stop_nrt_profile(str(output_dir).encode())

                hook = _hook
        except OSError:
            pass

    mod = types.ModuleType("antenv.axon_hooks")
    mod.get_axon_ntff_profile_hook = lambda: hook
    mod.set_axon_ntff_profile_hook = lambda h: None
    sys.modules["antenv.axon_hooks"] = mod

    import concourse.bass_utils as _bu

    _bu.upload_artifacts = lambda tmpdir: tmpdir


E4M3 = np.dtype(ml_dtypes.float8_e4m3)

NCORES = 8
B = 65536
BL = B // NCORES          # 8192 rows per core
D0, H1, H2, DO = 784, 400, 200, 10
CH = 512                  # batch columns per chunk (PSUM bank = 512 fp32)
NCH = BL // CH            # 16 chunks per core
GRP = 4                   # chunks per m4 packing group
NG = NCH // GRP
KH = 3                    # hi-pass DoubleRow k-tiles (K=256 each, 768 rows)
KL = 6                    # lo-pass fp16 k-tiles (K=128 each, 768 rows)
NWARM = 280               # PE warmup matmuls (HAM ramp + DMA-wait cover)

_cache = {}


def _build():
    if "nc" in _cache:
        return _cache["nc"]

    f32 = mybir.dt.float32
    f16 = mybir.dt.float16
    f8 = mybir.dt.float8e4
    DR = mybir.MatmulPerfMode.DoubleRow
    Sign = mybir.ActivationFunctionType.Sign

    nc = bacc.Bacc("TRN2", debug=False, num_devices=NCORES)

    # x hi plane: [chunk][p, k, pair, col], K-row r = 256k + 128i + p
    d_xhi = nc.dram_tensor("xhi", [NCH, 128, KH, 2, CH], f8, kind="ExternalInput").ap()
    # x lo plane + tail: k 0:6 lo (r = 128k + p); k=6 = 32-row fp16 tail
    # (hi[768:784] ++ lo[768:784]) replicated at partition strips 0/32/64
    d_xlt = nc.dram_tensor("xlt", [NCH, 128, KL + 1, CH], f16, kind="ExternalInput").ap()
    d_w1h = nc.dram_tensor("w1h", [128, KH, 2, H1], f8, kind="ExternalInput").ap()
    d_w1l = nc.dram_tensor("w1l", [128, KL + 1, H1], f16, kind="ExternalInput").ap()
    d_w2a = nc.dram_tensor("w2a", [128, 2, 256], f8, kind="ExternalInput").ap()
    # w2b[jj]: pair 0 = W2 rows 256:384; pair 1 = rows 384:400 at partition
    # strip 32jj (zeros elsewhere, masking the other chunks' packed signs)
    d_w2b = nc.dram_tensor("w2b", [128, GRP, 2, 256], f8, kind="ExternalInput").ap()
    d_w3 = nc.dram_tensor("w3", [128, 2, 16], f8, kind="ExternalInput").ap()
    d_out = nc.dram_tensor("out", [NCH, DO, CH], f32, kind="ExternalOutput").ap()

    with tile.TileContext(nc) as tc:
        with (
            tc.tile_pool(name="wp", bufs=1) as wp,
            tc.tile_pool(name="xp", bufs=8) as xp,
            tc.tile_pool(name="ap_", bufs=2) as apool,
            tc.tile_pool(name="a2p", bufs=2) as a2pool,
            tc.tile_pool(name="op", bufs=2) as op,
            tc.tile_pool(name="ps1p", bufs=1, space="PSUM") as ps1p,
            tc.tile_pool(name="ps2p", bufs=1, space="PSUM") as ps2p,
            tc.tile_pool(name="pspk", bufs=2, space="PSUM") as pspk,
        ):
            w1h = wp.tile([128, KH, 2, H1], f8, name="w1h")
            w1l = wp.tile([128, KL + 1, H1], f16, name="w1l")
            w2a = wp.tile([128, 2, 256], f8, name="w2a")
            w2b = wp.tile([128, GRP, 2, 256], f8, name="w2b")
            w3 = wp.tile([128, 2, 16], f8, name="w3")
            nc.sync.dma_start(out=w1h[:], in_=d_w1h)

            # HAM/P-state pre-warm: dummy matmuls keep the PE busy during the
            # initial weight/x DMA wait so the first real matmuls run at full
            # clock (the activity window is ~3.4us).
            warm = wp.tile([128, 64], f16, name="warm")
            nc.vector.memset(warm[:], 1.0)
            wps = pspk.tile([64, 64], f32, name="wps", tag="pack")
            for _ in range(NWARM):
                nc.tensor.matmul(wps[:], warm[:, 0:64], warm[:], start=True, stop=True)

            # zero the a2 pair-1 garbage partitions once per rotating buffer
            # (w3 is also zero there; this guards against NaN/Inf x 0)
            for _ in range(2):
                a2z = a2pool.tile([128, 2, CH], f8, name="a2")
                nc.vector.memset(a2z[64:128, 1, :], 0.0)

            def layer1(xhi, xlt, last=False):
                """Full-width layer-1 m-tiles; returns (a1p, a1x).

                a1p[:, 0/1, :] = sign(h1) features 0:128 / 128:256 (e4m3);
                a1x[:, 0, :] = features 256:384 (pair 1 filled later with the
                packed m4 signs)."""
                pss = []
                for m in range(3):
                    ps = ps1p.tile(
                        [128, CH], f32, name=f"ps1_{m}", bufs=(2 if m == 0 else 1)
                    )
                    for k in range(KH):
                        nc.tensor.matmul(
                            ps[:],
                            w1h[:, k, :, m * 128 : (m + 1) * 128],
                            xhi[:, k, :, :],
                            start=(k == 0),
                            stop=False,
                            perf_mode=DR,
                        )
                    for k in range(KL):
                        nc.tensor.matmul(
                            ps[:],
                            w1l[:, k, m * 128 : (m + 1) * 128],
                            xlt[:, k, :],
                            start=False,
                            stop=False,
                        )
                    if last:
                        # tail immediately per m-tile so Sign fires early and
                        # the final L2/L3 chain shortens the kernel epilogue
                        s = 32 * m
                        nc.tensor.matmul(
                            ps[:],
                            w1l[s : s + 32, KL, m * 128 : (m + 1) * 128],
                            xlt[s : s + 32, KL, :],
                            start=False,
                            stop=True,
                        )
                    pss.append(ps)
                if not last:
                    # shared 32-row fp16 tail window: 3 m-tiles at partition
                    # strips 0/32/64 run concurrently in distinct row groups
                    for m in range(3):
                        s = 32 * m
                        nc.tensor.matmul(
                            pss[m][:],
                            w1l[s : s + 32, KL, m * 128 : (m + 1) * 128],
                            xlt[s : s + 32, KL, :],
                            start=False,
                            stop=True,
                        )
                a1p = apool.tile([128, 2, CH], f8, name="a1p")
                a1x = apool.tile([128, 2, CH], f8, name="a1x")
                nc.scalar.activation(a1p[:, 0, :], pss[0][:], Sign)
                nc.scalar.activation(a1p[:, 1, :], pss[1][:], Sign)
                nc.scalar.activation(a1x[:, 0, :], pss[2][:], Sign)
                return a1p, a1x

            def m4_group(xhis, xlts):
                """Packed m4 (features 384:400) for 4 chunks: matmuls at col
                strips 0/32/64/96 run concurrently. Returns a13p (e4m3)."""
                ps4 = pspk.tile([128, CH], f32, name="ps4", tag="pack")
                nc.vector.memset(ps4[:], 0.0)
                for k in range(KH):
                    for i in range(2):
                        for jj in range(GRP):
                            s = 32 * jj
                            nc.tensor.matmul(
                                ps4[s : s + 16, :],
                                w1h[:, k, i, 384:400],
                                xhis[jj][:, k, i, :],
                                start=False,
                                stop=False,
                                tile_position=(0, s),
                            )
                for k in range(KL):
                    for jj in range(GRP):
                        s = 32 * jj
                        nc.tensor.matmul(
                            ps4[s : s + 16, :],
                            w1l[:, k, 384:400],
                            xlts[jj][:, k, :],
                            start=False,
                            stop=False,
                            tile_position=(0, s),
                        )
                for jj in range(GRP):
                    s = 32 * jj
                    nc.tensor.matmul(
                        ps4[s : s + 16, :],
                        w1l[0:32, KL, 384:400],
                        xlts[jj][0:32, KL, :],
                        start=False,
                        stop=(jj == GRP - 1),
                        tile_position=(0, s),
                    )
                a13p = apool.tile([128, CH], f8, name="a13p")
                nc.scalar.activation(a13p[:], ps4[:], Sign)
                return a13p

            def layer2(jj, a1p, a1x):
                """Layer 2: two DoubleRow matmuls per m-tile; returns a2."""
                a2 = a2pool.tile([128, 2, CH], f8, name="a2")
                pss2 = []
                for m, msz in ((0, 128), (1, 72)):
                    mo = m * 128
                    ps = ps2p.tile([msz, CH], f32, name=f"ps2_{m}")
                    nc.tensor.matmul(
                        ps[:],
                        w2a[:, :, mo : mo + msz],
                        a1p[:],
                        start=True,
                        stop=False,
                        perf_mode=DR,
                    )
                    nc.tensor.matmul(
                        ps[:],
                        w2b[:, jj, :, mo : mo + msz],
                        a1x[:],
                        start=False,
                        stop=True,
                        perf_mode=DR,
                    )
                    pss2.append(ps)
                nc.scalar.activation(a2[:, 0, :], pss2[0][:], Sign)
                nc.scalar.activation(a2[0:72, 1, :], pss2[1][:], Sign)
                return a2

            def layer3(c, a2):
                ps3 = pspk.tile([128, CH], f32, name="ps3", tag="pack")
                nc.tensor.matmul(
                    ps3[0:DO, :],
                    w3[:, :, 0:DO],
                    a2[:],
                    start=True,
                    stop=True,
                    perf_mode=DR,
                )
                osb = op.tile([16, CH], f32, name="osb")
                nc.vector.tensor_copy(osb[0:DO, :], ps3[0:DO, :])
                nc.sync.dma_start(out=d_out[c], in_=osb[0:DO, :])

            def dma_group(g):
                xhis, xlts = [], []
                for jj in range(GRP):
                    c = g * GRP + jj
                    xhi = xp.tile([128, KH, 2, CH], f8, name="xhi")
                    xlt = xp.tile([128, KL + 1, CH], f16, name="xlt")
                    if g == 0 and jj == 0:
                        # split the first chunk's transfers so the PE can
                        # start on partial data as it lands
                        nc.sync.dma_start(out=xhi[:, 0, :, :], in_=d_xhi[c, :, 0])
                        nc.sync.dma_start(out=xhi[:, 1:KH, :, :], in_=d_xhi[c, :, 1:KH])
                        nc.sync.dma_start(out=w1l[:], in_=d_w1l)
                        nc.sync.dma_start(out=xlt[:, 0:3, :], in_=d_xlt[c, :, 0:3])
                        nc.sync.dma_start(out=xlt[:, 3 : KL + 1, :], in_=d_xlt[c, :, 3 : KL + 1])
                    else:
                        nc.sync.dma_start(out=xhi[:], in_=d_xhi[c])
                        nc.sync.dma_start(out=xlt[:], in_=d_xlt[c])
                    xhis.append(xhi)
                    xlts.append(xlt)
                    if g == 0 and jj == 1:
                        nc.sync.dma_start(out=w2a[:], in_=d_w2a)
                        nc.sync.dma_start(out=w2b[:], in_=d_w2b)
                        nc.sync.dma_start(out=w3[:], in_=d_w3)
                return xhis, xlts

            # -------- software-pipelined emission over the 16 chunks --------
            # L1(c) leads; m4(G) after L1 of G's second chunk; L2 lags one
            # chunk behind L1, L3 two chunks behind; DVE copies a13p into each
            # chunk's a1x pair 1.
            xs = {}
            a1 = {}
            a2 = {}
            a13 = {}

            def do_L1(c, last=False):
                a1[c] = layer1(xs[c][0], xs[c][1], last=last)
                g = c // GRP
                if g in a13:
                    # m4 for this group already done: fill pair 1 now so the
                    # copy is off the L2 critical path
                    nc.vector.tensor_copy(a1[c][1][:, 1, :], a13[g][:])

            def do_m4(g):
                cs = [g * GRP + j for j in range(GRP)]
                a13[g] = m4_group([xs[c][0] for c in cs], [xs[c][1] for c in cs])
                for c in cs[:2]:
                    nc.vector.tensor_copy(a1[c][1][:, 1, :], a13[g][:])

            def do_L2(c):
                a2[c] = layer2(c % GRP, *a1[c])

            def do_L3(c):
                layer3(c, a2[c])

            def prefetch(g):
                if g < NG:
                    xh, xl = dma_group(g)
                    for j in range(GRP):
                        xs[g * GRP + j] = (xh[j], xl[j])

            prefetch(0)
            do_L1(0)
            prefetch(1)
            do_L1(1)
            do_m4(0)
            do_L2(0)
            for c in range(2, NCH):
                g, jj = divmod(c, GRP)
                if jj == 1:
                    prefetch(g + 1)
                do_L1(c, last=(c == NCH - 1))
                if jj == 1:
                    do_m4(g)
                do_L2(c - 1)
                do_L3(c - 2)
            do_L2(NCH - 1)
            do_L3(NCH - 2)
            do_L3(NCH - 1)

    nc.compile()
    _cache["nc"] = nc
    return nc


def _prep_weights(W1, W2, W3):
    s1 = np.sign(W1).T.astype(np.float32)  # [784, 400]
    w1h = np.ascontiguousarray(
        s1[:768].reshape(KH, 2, 128, H1).transpose(2, 0, 1, 3)
    ).astype(E4M3)  # [128, 3, 2, 400]
    w1l = np.zeros((128, KL + 1, H1), np.float16)
    w1l[:, 0:KL, :] = s1[:768].reshape(KL, 128, H1).transpose(1, 0, 2)
    trip = np.concatenate([s1[768:784], s1[768:784]], axis=0)  # [32, 400]
    for m in range(3):
        w1l[32 * m : 32 * m + 32, KL, :] = trip

    s2 = np.sign(W2).T.astype(np.float32)  # [400, 200]
    w2a = np.zeros((128, 2, 256), np.float32)
    w2a[:, 0, 0:H2] = s2[0:128]
    w2a[:, 1, 0:H2] = s2[128:256]
    w2b = np.zeros((128, GRP, 2, 256), np.float32)
    for jj in range(GRP):
        w2b[:, jj, 0, 0:H2] = s2[256:384]
        w2b[32 * jj : 32 * jj + 16, jj, 1, 0:H2] = s2[384:400]

    s3 = np.sign(W3).T.astype(np.float32)  # [200, 10]
    w3 = np.zeros((128, 2, 16), np.float32)
    w3[:, 0, 0:DO] = s3[0:128]
    w3[0:72, 1, 0:DO] = s3[128:200]

    return w1h, w1l, w2a.astype(E4M3), w2b.astype(E4M3), w3.astype(E4M3)


def _prep_x_core(xc):
    # xc: [8192, 784] fp32 -> hi e4m3 [16, 128, 3, 2, 512], lo+tail fp16
    # [16, 128, 7, 512]
    xt = np.ascontiguousarray(xc.T.astype(np.float32))  # [784, 8192]
    hi8 = xt.astype(E4M3)
    lo = (xt - hi8.astype(np.float32)).astype(np.float16)  # [784, 8192]
    xhi = np.ascontiguousarray(
        hi8[:768].reshape(KH, 2, 128, NCH, CH).transpose(3, 2, 0, 1, 4)
    )  # [16, 128, 3, 2, 512]
    xlt = np.zeros((NCH, 128, KL + 1, CH), np.float16)
    xlt[:, :, 0:KL, :] = lo[:768].reshape(KL, 128, NCH, CH).transpose(2, 1, 0, 3)
    hi16 = hi8[768:784].astype(np.float16)  # exact
    tail = np.zeros((128, BL), np.float16)
    for m in range(3):
        tail[32 * m : 32 * m + 16] = hi16
        tail[32 * m + 16 : 32 * m + 32] = lo[768:784]
    xlt[:, :, KL, :] = tail.reshape(128, NCH, CH).transpose(1, 0, 2)
    return xhi, np.ascontiguousarray(xlt)


def kernel(x, W1, W2, W3, _trace=False, **_kw):
    nc = _build()
    w1h, w1l, w2a, w2b, w3 = _prep_weights(
        np.asarray(W1, np.float32), np.asarray(W2, np.float32), np.asarray(W3, np.float32)
    )
    x = np.asarray(x, np.float32).reshape(B, D0)

    in_maps = []
    for c in range(NCORES):
        xhi, xlt = _prep_x_core(x[c * BL : (c + 1) * BL])
        in_maps.append(
            {
                "xhi": xhi,
                "xlt": xlt,
                "w1h": w1h,
                "w1l": w1l,
                "w2a": w2a,
                "w2b": w2b,
                "w3": w3,
            }
        )

    _ensure_axon_hooks()
    res = run_bass_kernel_spmd(nc, in_maps, core_ids=list(range(NCORES)), trace=_trace)

    out = np.empty((B, DO), np.float32)
    for c in range(NCORES):
        oc = res.results[c]["out"]  # [16, 10, 512]
        out[c * BL : (c + 1) * BL] = oc.transpose(0, 2, 1).reshape(BL, DO)
    if _trace:
        _cache["last_results"] = res
    return out


# revision 12
# speedup vs baseline: 1.3710x; 1.0143x over previous
"""Binarized 3-layer MLP on 8 TRN2 NeuronCores (data-parallel over batch).

Computation (matching the reference):
    h1  = x @ sign(W1).T          x: [65536, 784] fp32, W1: [400, 784]
    h2  = sign(h1) @ sign(W2).T   W2: [200, 400]
    out = sign(h2) @ sign(W3).T   W3: [10, 200]

Strategy (v3):
  - Batch sharded 8192 rows/core; weights replicated.
  - Layer 1 precision: x split as hi = e4m3(x) plus lo = fp16(x - hi).
    The hi pass runs as fp8 DoubleRow matmuls (2 K-rows per PE cell,
    K=256 per matmul); the lo pass runs as fp16 matmuls. Combined
    representation error ~2^-15 relative -> rel err ~8e-3 via sign-flip
    propagation, vs the 2e-2 gate.
  - K tail (rows 768:784): hi(e4m3->fp16 exact) and lo stacked as one
    32-row fp16 tile, replicated at partition strips 0/32/64 so the
    three m-tiles' tail matmuls run concurrently in one window.
  - m4 (h1 features 384:400): col-packed, groups of 4 chunks run their
    m4 matmuls in distinct 32-column PE groups concurrently.
  - Layer 2 (K=400): two fp8 DoubleRow matmuls per m-tile. The second
    pairs features 256:384 (a1x pair 0) with the packed m4 signs
    (a1x pair 1 = copy of a13p); per-chunk-position weight tiles w2b[jj]
    zero out all but that chunk's 16-row strip.
  - Layer 3 (K=200): one fp8 DoubleRow matmul per chunk; the a2 pair
    tile holds features 0:128 at pair 0 and 128:200 at pair 1 with
    zeroed weights over the 72:128 garbage partitions.
  - Sign() outputs are exactly representable in e4m3, and PSUM
    accumulates fp32, so layers 2/3 are exact.
  - Issue schedule is software-pipelined: L2 lags L1 by one chunk and
    L3 by two, so Sign (scalar engine) latency is hidden under the next
    chunk's matmul streams, across group boundaries.
"""

import contextlib
import ctypes
import os
import sys
import types

import numpy as np
import ml_dtypes

import concourse.bacc as bacc
import concourse.mybir as mybir
import concourse.tile as tile
from concourse.bass_utils import run_bass_kernel_spmd


def _ensure_axon_hooks():
    """concourse's trace path imports antenv.axon_hooks, which this image
    lacks; register a ctypes-backed stand-in so trace=True (or a stray
    BASS_TRACE=1 in the environment) cannot crash the run."""
    try:
        import antenv.axon_hooks  # noqa: F401
        return
    except ImportError:
        pass

    so_path = "/opt/axon/libaxon_pjrt.so"
    hook = None
    if os.path.exists(so_path):
        try:
            lib = ctypes.CDLL(so_path)
            if hasattr(lib, "axon_start_nrt_profile"):
                lib.axon_start_nrt_profile.argtypes = [
                    ctypes.POINTER(ctypes.c_int64),
                    ctypes.c_size_t,
                ]
                lib.axon_start_nrt_profile.restype = ctypes.c_int64
                lib.axon_stop_nrt_profile.argtypes = [ctypes.c_char_p]
                lib.axon_stop_nrt_profile.restype = ctypes.c_int64

                @contextlib.contextmanager
                def _hook(output_dir, device_ids):
                    import jax

                    jax.devices()
                    if device_ids:
                        ids = (ctypes.c_int64 * len(device_ids))(*device_ids)
                        rc = lib.axon_start_nrt_profile(ids, len(device_ids))
                    else:
                        rc = lib.axon_start_nrt_profile(None, 0)
                    if rc != 0:
                        raise RuntimeError(f"axon_start_nrt_profile rc={rc}")
                    try:
                        yield
                    finally:
                        lib.axon_stop_nrt_profile(str(output_dir).encode())

                hook = _hook
        except OSError:
            pass

    mod = types.ModuleType("antenv.axon_hooks")
    mod.get_axon_ntff_profile_hook = lambda: hook
    mod.set_axon_ntff_profile_hook = lambda h: None
    sys.modules["antenv.axon_hooks"] = mod

    import concourse.bass_utils as _bu

    _bu.upload_artifacts = lambda tmpdir: tmpdir


E4M3 = np.dtype(ml_dtypes.float8_e4m3)

NCORES = 8
B = 65536
BL = B // NCORES          # 8192 rows per core
D0, H1, H2, DO = 784, 400, 200, 10
CH = 512                  # batch columns per chunk (PSUM bank = 512 fp32)
NCH = BL // CH            # 16 chunks per core
GRP = 4                   # chunks per m4 packing group
NG = NCH // GRP
KH = 3                    # hi-pass DoubleRow k-tiles (K=256 each, 768 rows)
KL = 6                    # lo-pass fp16 k-tiles (K=128 each, 768 rows)
NWARM = 225               # PE warmup matmuls (HAM ramp + DMA-wait cover)

_cache = {}


def _build():
    if "nc" in _cache:
        return _cache["nc"]

    f32 = mybir.dt.float32
    f16 = mybir.dt.float16
    f8 = mybir.dt.float8e4
    DR = mybir.MatmulPerfMode.DoubleRow
    Sign = mybir.ActivationFunctionType.Sign

    nc = bacc.Bacc("TRN2", debug=False, num_devices=NCORES)

    # x hi plane: [chunk][p, k, pair, col], K-row r = 256k + 128i + p
    d_xhi = nc.dram_tensor("xhi", [NCH, 128, KH, 2, CH], f8, kind="ExternalInput").ap()
    # x lo plane + tail: k 0:6 lo (r = 128k + p); k=6 = 32-row fp16 tail
    # (hi[768:784] ++ lo[768:784]) replicated at partition strips 0/32/64
    d_xlt = nc.dram_tensor("xlt", [NCH, 128, KL + 1, CH], f16, kind="ExternalInput").ap()
    d_w1h = nc.dram_tensor("w1h", [128, KH, 2, H1], f8, kind="ExternalInput").ap()
    d_w1l = nc.dram_tensor("w1l", [128, KL + 1, H1], f16, kind="ExternalInput").ap()
    d_w2a = nc.dram_tensor("w2a", [128, 2, 256], f8, kind="ExternalInput").ap()
    # w2b[jj]: pair 0 = W2 rows 256:384; pair 1 = rows 384:400 at partition
    # strip 32jj (zeros elsewhere, masking the other chunks' packed signs)
    d_w2b = nc.dram_tensor("w2b", [128, GRP, 2, 256], f8, kind="ExternalInput").ap()
    d_w3 = nc.dram_tensor("w3", [128, 2, 16], f8, kind="ExternalInput").ap()
    d_out = nc.dram_tensor("out", [NCH, DO, CH], f32, kind="ExternalOutput").ap()

    with tile.TileContext(nc) as tc:
        with (
            tc.tile_pool(name="wp", bufs=1) as wp,
            tc.tile_pool(name="xp", bufs=8) as xp,
            tc.tile_pool(name="ap_", bufs=2) as apool,
            tc.tile_pool(name="a2p", bufs=2) as a2pool,
            tc.tile_pool(name="op", bufs=2) as op,
            tc.tile_pool(name="ps1p", bufs=1, space="PSUM") as ps1p,
            tc.tile_pool(name="ps2p", bufs=1, space="PSUM") as ps2p,
            tc.tile_pool(name="pspk", bufs=2, space="PSUM") as pspk,
        ):
            w1h = wp.tile([128, KH, 2, H1], f8, name="w1h")
            w1l = wp.tile([128, KL + 1, H1], f16, name="w1l")
            w2a = wp.tile([128, 2, 256], f8, name="w2a")
            w2b = wp.tile([128, GRP, 2, 256], f8, name="w2b")
            w3 = wp.tile([128, 2, 16], f8, name="w3")
            nc.sync.dma_start(out=w1h[:], in_=d_w1h)

            # HAM/P-state pre-warm: dummy matmuls keep the PE busy during the
            # initial weight/x DMA wait so the first real matmuls run at full
            # clock (the activity window is ~3.4us).
            warm = wp.tile([128, 64], f16, name="warm")
            nc.vector.memset(warm[:], 1.0)
            wps = pspk.tile([64, 64], f32, name="wps", tag="pack")
            for _ in range(NWARM):
                nc.tensor.matmul(wps[:], warm[:, 0:64], warm[:], start=True, stop=True)

            # zero the a2 pair-1 garbage partitions once per rotating buffer
            # (w3 is also zero there; this guards against NaN/Inf x 0)
            for _ in range(2):
                a2z = a2pool.tile([128, 2, CH], f8, name="a2")
                nc.vector.memset(a2z[64:128, 1, :], 0.0)

            def layer1_mms(xhi, xlt, last=False):
                """Full-width layer-1 m-tile matmuls; returns psum tiles."""
                pss = []
                for m in range(3):
                    ps = ps1p.tile(
                        [128, CH], f32, name=f"ps1_{m}", bufs=(2 if m == 0 else 1)
                    )
                    for k in range(KH):
                        nc.tensor.matmul(
                            ps[:],
                            w1h[:, k, :, m * 128 : (m + 1) * 128],
                            xhi[:, k, :, :],
                            start=(k == 0),
                            stop=False,
                            perf_mode=DR,
                        )
                    for k in range(KL):
                        nc.tensor.matmul(
                            ps[:],
                            w1l[:, k, m * 128 : (m + 1) * 128],
                            xlt[:, k, :],
                            start=False,
                            stop=False,
                        )
                    if last:
                        # tail immediately per m-tile so Sign fires early and
                        # the final L2/L3 chain shortens the kernel epilogue
                        s = 32 * m
                        nc.tensor.matmul(
                            ps[:],
                            w1l[s : s + 32, KL, m * 128 : (m + 1) * 128],
                            xlt[s : s + 32, KL, :],
                            start=False,
                            stop=True,
                        )
                    pss.append(ps)
                if not last:
                    # shared 32-row fp16 tail window: 3 m-tiles at partition
                    # strips 0/32/64 run concurrently in distinct row groups
                    for m in range(3):
                        s = 32 * m
                        nc.tensor.matmul(
                            pss[m][:],
                            w1l[s : s + 32, KL, m * 128 : (m + 1) * 128],
                            xlt[s : s + 32, KL, :],
                            start=False,
                            stop=True,
                        )
                return pss

            def layer1_acts(pss):
                """Sign activations for layer 1; a1p pairs features 0:128 and
                128:256, a1x pair 0 = features 256:384 (pair 1 filled with the
                packed m4 signs separately)."""
                a1p = apool.tile([128, 2, CH], f8, name="a1p")
                a1x = apool.tile([128, 2, CH], f8, name="a1x")
                nc.scalar.activation(a1p[:, 0, :], pss[0][:], Sign)
                nc.scalar.activation(a1p[:, 1, :], pss[1][:], Sign)
                nc.scalar.activation(a1x[:, 0, :], pss[2][:], Sign)
                return a1p, a1x

            def m4_group(xhis, xlts):
                """Packed m4 (features 384:400) for 4 chunks: matmuls at col
                strips 0/32/64/96 run concurrently. Returns a13p (e4m3)."""
                ps4 = pspk.tile([128, CH], f32, name="ps4", tag="pack")
                nc.vector.memset(ps4[:], 0.0)
                for k in range(KH):
                    for i in range(2):
                        for jj in range(GRP):
                            s = 32 * jj
                            nc.tensor.matmul(
                                ps4[s : s + 16, :],
                                w1h[:, k, i, 384:400],
                                xhis[jj][:, k, i, :],
                                start=False,
                                stop=False,
                                tile_position=(0, s),
                            )
                for k in range(KL):
                    for jj in range(GRP):
                        s = 32 * jj
                        nc.tensor.matmul(
                            ps4[s : s + 16, :],
                            w1l[:, k, 384:400],
                            xlts[jj][:, k, :],
                            start=False,
                            stop=False,
                            tile_position=(0, s),
                        )
                for jj in range(GRP):
                    s = 32 * jj
                    nc.tensor.matmul(
                        ps4[s : s + 16, :],
                        w1l[0:32, KL, 384:400],
                        xlts[jj][0:32, KL, :],
                        start=False,
                        stop=(jj == GRP - 1),
                        tile_position=(0, s),
                    )
                a13p = apool.tile([128, CH], f8, name="a13p")
                nc.scalar.activation(a13p[:], ps4[:], Sign)
                return a13p

            def layer2_mms(jj, a1p, a1x):
                """Layer 2: two DoubleRow matmuls per m-tile. DR#1s first so
                the DR#2s (which need the a13p copy) get extra stream cover."""
                pss2 = []
                for m, msz in ((0, 128), (1, 72)):
                    mo = m * 128
                    ps = ps2p.tile([msz, CH], f32, name=f"ps2_{m}")
                    nc.tensor.matmul(
                        ps[:],
                        w2a[:, :, mo : mo + msz],
                        a1p[:],
                        start=True,
                        stop=False,
                        perf_mode=DR,
                    )
                    pss2.append(ps)
                for m, msz in ((0, 128), (1, 72)):
                    mo = m * 128
                    nc.tensor.matmul(
                        pss2[m][:],
                        w2b[:, jj, :, mo : mo + msz],
                        a1x[:],
                        start=False,
                        stop=True,
                        perf_mode=DR,
                    )
                return pss2

            def layer2_acts(pss2):
                a2 = a2pool.tile([128, 2, CH], f8, name="a2")
                nc.scalar.activation(a2[:, 0, :], pss2[0][:], Sign)
                nc.scalar.activation(a2[0:72, 1, :], pss2[1][:], Sign)
                return a2

            def layer3(c, a2):
                ps3 = pspk.tile([128, CH], f32, name="ps3", tag="pack")
                nc.tensor.matmul(
                    ps3[0:DO, :],
                    w3[:, :, 0:DO],
                    a2[:],
                    start=True,
                    stop=True,
                    perf_mode=DR,
                )
                osb = op.tile([16, CH], f32, name="osb")
                nc.vector.tensor_copy(osb[0:DO, :], ps3[0:DO, :])
                nc.sync.dma_start(out=d_out[c], in_=osb[0:DO, :])

            def dma_group(g):
                xhis, xlts = [], []
                for jj in range(GRP):
                    c = g * GRP + jj
                    xhi = xp.tile([128, KH, 2, CH], f8, name="xhi")
                    xlt = xp.tile([128, KL + 1, CH], f16, name="xlt")
                    if g == 0 and jj == 0:
                        # split the first chunk's transfers so the PE can
                        # start on partial data as it lands
                        nc.sync.dma_start(out=xhi[:, 0, :, :], in_=d_xhi[c, :, 0])
                        nc.sync.dma_start(out=xhi[:, 1:KH, :, :], in_=d_xhi[c, :, 1:KH])
                        nc.sync.dma_start(out=w1l[:, 0:3, :], in_=d_w1l[:, 0:3])
                        nc.sync.dma_start(out=xlt[:, 0:3, :], in_=d_xlt[c, :, 0:3])
                        nc.sync.dma_start(out=w1l[:, 3 : KL + 1, :], in_=d_w1l[:, 3 : KL + 1])
                        nc.sync.dma_start(out=xlt[:, 3 : KL + 1, :], in_=d_xlt[c, :, 3 : KL + 1])
                    else:
                        nc.sync.dma_start(out=xhi[:], in_=d_xhi[c])
                        nc.sync.dma_start(out=xlt[:], in_=d_xlt[c])
                    xhis.append(xhi)
                    xlts.append(xlt)
                    if g == 0 and jj == 1:
                        nc.sync.dma_start(out=w2a[:], in_=d_w2a)
                        nc.sync.dma_start(out=w2b[:], in_=d_w2b)
                        nc.sync.dma_start(out=w3[:], in_=d_w3)
                return xhis, xlts

            # -------- software-pipelined emission over the 16 chunks --------
            # Per steady step c (PE order): L1-matmuls(c), [m4-matmuls(G) on
            # group steps], L2-matmuls(c-1), L3(c-2). Scalar order within the
            # step: a2-acts(c-1) BEFORE a1-acts(c) so the ps2 banks free up
            # one chunk ahead of their reuse; a13p act right after m4 so the
            # DVE copies into a1x pair 1 are ready for L2 of the group.
            xs = {}
            ps1 = {}
            ps2 = {}
            a1 = {}
            a2 = {}
            a13 = {}

            def do_m4(g):
                cs = [g * GRP + j for j in range(GRP)]
                a13[g] = m4_group([xs[c][0] for c in cs], [xs[c][1] for c in cs])
                for c in cs:
                    if c in a1:
                        nc.vector.tensor_copy(a1[c][1][:, 1, :], a13[g][:])

            def do_a1_acts(c):
                a1[c] = layer1_acts(ps1.pop(c))
                g = c // GRP
                if g in a13:
                    nc.vector.tensor_copy(a1[c][1][:, 1, :], a13[g][:])

            def prefetch(g):
                if g < NG:
                    xh, xl = dma_group(g)
                    for j in range(GRP):
                        xs[g * GRP + j] = (xh[j], xl[j])

            prefetch(0)
            ps1[0] = layer1_mms(*xs[0])
            prefetch(1)
            do_a1_acts(0)
            ps1[1] = layer1_mms(*xs[1])
            do_m4(0)
            do_a1_acts(1)
            ps2[0] = layer2_mms(0, *a1[0])
            a2[0] = layer2_acts(ps2.pop(0))
            for c in range(2, NCH):
                g, jj = divmod(c, GRP)
                if jj == 1:
                    prefetch(g + 1)
                ps1[c] = layer1_mms(*xs[c], last=(c == NCH - 1))
                if jj == 0:
                    do_m4(g)
                ps2[c - 1] = layer2_mms((c - 1) % GRP, *a1[c - 1])
                a2[c - 1] = layer2_acts(ps2.pop(c - 1))
                do_a1_acts(c)
                layer3(c - 2, a2.pop(c - 2))
            ps2[NCH - 1] = layer2_mms((NCH - 1) % GRP, *a1[NCH - 1])
            a2[NCH - 1] = layer2_acts(ps2.pop(NCH - 1))
            layer3(NCH - 2, a2.pop(NCH - 2))
            layer3(NCH - 1, a2.pop(NCH - 1))

    nc.compile()
    _cache["nc"] = nc
    return nc


def _prep_weights(W1, W2, W3):
    s1 = np.sign(W1).T.astype(np.float32)  # [784, 400]
    w1h = np.ascontiguousarray(
        s1[:768].reshape(KH, 2, 128, H1).transpose(2, 0, 1, 3)
    ).astype(E4M3)  # [128, 3, 2, 400]
    w1l = np.zeros((128, KL + 1, H1), np.float16)
    w1l[:, 0:KL, :] = s1[:768].reshape(KL, 128, H1).transpose(1, 0, 2)
    trip = np.concatenate([s1[768:784], s1[768:784]], axis=0)  # [32, 400]
    for m in range(3):
        w1l[32 * m : 32 * m + 32, KL, :] = trip

    s2 = np.sign(W2).T.astype(np.float32)  # [400, 200]
    w2a = np.zeros((128, 2, 256), np.float32)
    w2a[:, 0, 0:H2] = s2[0:128]
    w2a[:, 1, 0:H2] = s2[128:256]
    w2b = np.zeros((128, GRP, 2, 256), np.float32)
    for jj in range(GRP):
        w2b[:, jj, 0, 0:H2] = s2[256:384]
        w2b[32 * jj : 32 * jj + 16, jj, 1, 0:H2] = s2[384:400]

    s3 = np.sign(W3).T.astype(np.float32)  # [200, 10]
    w3 = np.zeros((128, 2, 16), np.float32)
    w3[:, 0, 0:DO] = s3[0:128]
    w3[0:72, 1, 0:DO] = s3[128:200]

    return w1h, w1l, w2a.astype(E4M3), w2b.astype(E4M3), w3.astype(E4M3)


def _prep_x_core(xc):
    # xc: [8192, 784] fp32 -> hi e4m3 [16, 128, 3, 2, 512], lo+tail fp16
    # [16, 128, 7, 512]
    xt = np.ascontiguousarray(xc.T.astype(np.float32))  # [784, 8192]
    hi8 = xt.astype(E4M3)
    lo = (xt - hi8.astype(np.float32)).astype(np.float16)  # [784, 8192]
    xhi = np.ascontiguousarray(
        hi8[:768].reshape(KH, 2, 128, NCH, CH).transpose(3, 2, 0, 1, 4)
    )  # [16, 128, 3, 2, 512]
    xlt = np.zeros((NCH, 128, KL + 1, CH), np.float16)
    xlt[:, :, 0:KL, :] = lo[:768].reshape(KL, 128, NCH, CH).transpose(2, 1, 0, 3)
    hi16 = hi8[768:784].astype(np.float16)  # exact
    tail = np.zeros((128, BL), np.float16)
    for m in range(3):
        tail[32 * m : 32 * m + 16] = hi16
        tail[32 * m + 16 : 32 * m + 32] = lo[768:784]
    xlt[:, :, KL, :] = tail.reshape(128, NCH, CH).transpose(1, 0, 2)
    return xhi, np.ascontiguousarray(xlt)


def kernel(x, W1, W2, W3, _trace=False, **_kw):
    nc = _build()
    w1h, w1l, w2a, w2b, w3 = _prep_weights(
        np.asarray(W1, np.float32), np.asarray(W2, np.float32), np.asarray(W3, np.float32)
    )
    x = np.asarray(x, np.float32).reshape(B, D0)

    in_maps = []
    for c in range(NCORES):
        xhi, xlt = _prep_x_core(x[c * BL : (c + 1) * BL])
        in_maps.append(
            {
                "xhi": xhi,
                "xlt": xlt,
                "w1h": w1h,
                "w1l": w1l,
                "w2a": w2a,
                "w2b": w2b,
                "w3": w3,
            }
        )

    _ensure_axon_hooks()
    res = run_bass_kernel_spmd(nc, in_maps, core_ids=list(range(NCORES)), trace=_trace)

    out = np.empty((B, DO), np.float32)
    for c in range(NCORES):
        oc = res.results[c]["out"]  # [16, 10, 512]
        out[c * BL : (c + 1) * BL] = oc.transpose(0, 2, 1).reshape(BL, DO)
    if _trace:
        _cache["last_results"] = res
    return out
